# revision 1
# baseline (speedup 1.0000x reference)
# CrossAttention (B=2, S=2048, D=1024, H=16, dh=64) on 8 trn2 NeuronCores.
#
# Sharding: 32 (batch, head) units, 4 consecutive units per core (cores 0-3
# work on batch 0, cores 4-7 on batch 1). Each core receives its batch's
# hidden states pre-permuted to the on-chip [128, D/128, S] transposed
# layout, per-head slices of Wq/Wk/Wv/Wo (also pre-permuted); it returns a
# partial output y [2048, 1024] (its heads' contribution to the output
# projection). The host sums the four partials per batch and adds bo
# (tensor-parallel unshard of the output projection).
#
# Device algorithm (per core, 4 units = 2 pairs of heads), all matmuls
# fp32r (full PE rate at N>=256, fp32 accumulation). Triangular software
# pipeline over 512-wide s-block "rounds"; for round r:
#   - DMA hidden^T slice; project V (natural [s, feat] layout, stored as V'
#     with fused ones columns), K^T and Q^T (pair-packed [128, s]).
#   - run every attention cell (qb, kc) with max(qb, kc) == r; a cell is:
#     per pair, 4 key chunks of: S^T = K^T-chunk^T Q^T (two heads
#     row-packed, K=64 each), P^T = exp(S^T/8) on ACT (one [128,1024] op
#     covers both heads), O' += V'^T P^T (M=65: psum rows 0-63 unnormalized
#     out^T, row 64 = softmax denominator via the ones column); O' is
#     accumulated across rounds in SBUF.
#   - a cell per round is deferred and interleaved with the next round's
#     projections so the ACT engine stays fed.
# Finalize per q-block (interleaved with the last round's cells):
# normalize rows 0-63 by 1/(row 64) (DVE reciprocal -> K=1 ones-matmul
# partition broadcast -> DVE multiply), then output projection
# y += O_u^T Wo_u (K=64 per unit) and DMA out.
#
# PSUM plan (8 banks): S^T tiles 2x[128,1024], PV accumulators 2x[128,512],
# projection/broadcast/output transients 2x[128,512] - dedicated pools so
# the streams don't steal each other's slots.
import os
import sys

import numpy as np

try:
    import concourse.bass as bass
except ImportError:  # harness runs from a fresh dir; repo is on the default path
    sys.path.insert(0, "/opt/trn_rl_repo")
    import concourse.bass as bass

import concourse.bacc as bacc
import concourse.mybir as mybir
import concourse.tile as tile
from concourse.bass import ts, ds
from contextlib import ExitStack

B, S, D = 2, 2048, 1024
HEADS, DIM_HEAD = 16, 64
SCALE = DIM_HEAD**-0.5
N_CORES = 8
UNITS = 4  # (b, h) units per core
PAIRS = 2  # head pairs per core
P = 128
SB = S // 512  # 4 s-blocks of 512
QB = S // 512  # 4 q-blocks of 512
DC = D // P  # 8 contraction chunks for projections
KI = S // P  # 16 key chunks of 128
F32 = mybir.dt.float32
F32R = mybir.dt.float32r


def build_nc():
    nc = bacc.Bacc("TRN2", target_bir_lowering=False, debug=False)

    hiddent = nc.dram_tensor("hiddent", [P, DC, S], F32R, kind="ExternalInput").ap()
    # weights arrive pre-permuted to the on-chip layout (see shard_inputs),
    # declared fp32r end-to-end so the PE takes the full-rate matmul path
    wqt = nc.dram_tensor("wqt", [P, DC, 256], F32R, kind="ExternalInput").ap()
    wkt = nc.dram_tensor("wkt", [P, DC, 256], F32R, kind="ExternalInput").ap()
    wvt = nc.dram_tensor("wvt", [P, DC, 256], F32R, kind="ExternalInput").ap()
    wot = nc.dram_tensor("wot", [64, UNITS, D], F32R, kind="ExternalInput").ap()
    y = nc.dram_tensor("y", [S, D], F32, kind="ExternalOutput").ap()

    with tile.TileContext(nc) as tc, ExitStack() as ctx:
        persist = ctx.enter_context(tc.tile_pool(name="persist", bufs=1))
        pt_pool = ctx.enter_context(
            tc.tile_pool(name="pt", bufs=int(os.environ.get("K_PT", "6")))
        )
        otsb_pool = ctx.enter_context(
            tc.tile_pool(name="otsb", bufs=int(os.environ.get("K_OTSB", "4")))
        )
        rc_pool = ctx.enter_context(tc.tile_pool(name="rc", bufs=2))
        y_pool = ctx.enter_context(tc.tile_pool(name="ysb", bufs=2))
        # PSUM: "st" [128,1024] x2 = 4 banks (hidden transposes + S^T tiles);
        # "ot" [128,512] x4 = 4 banks (projection accums, PV accums,
        # broadcast and output-projection accums).
        st_ps = ctx.enter_context(tc.tile_pool(name="stps", bufs=int(os.environ.get("K_ST", "2")), space="PSUM"))
        # PSUM split: S^T tiles 2x[128,1024] (4 banks), PV accumulators
        # 2x[128,512] (2 banks), projection/broadcast/output transients
        # 2x[128,512] (2 banks). Dedicated pools keep cells and transients
        # from stealing each other's slots.
        ot_ps = ctx.enter_context(
            tc.tile_pool(name="otps", bufs=int(os.environ.get("K_OT", "2")), space="PSUM")
        )
        cell_ps = ctx.enter_context(
            tc.tile_pool(name="cellps", bufs=int(os.environ.get("K_CELL", "2")), space="PSUM")
        )

        # ---- persistent SBUF tensors ----
        KT = persist.tile([P, PAIRS, S], F32R)  # K^T pair-packed
        QT = persist.tile([P, PAIRS, S], F32R)  # Q^T pair-packed
        # V' per (k-chunk, pair): [V_unitA(64) | 1 | V_unitB(64) | 1];
        # each unit's PV is M=65 at base 0: out rows 0-63, sum at row 64
        Vp = persist.tile([P, KI, PAIRS, 130], F32R)
        wq_sb = persist.tile([P, DC, 256], F32R)
        wk_sb = persist.tile([P, DC, 256], F32R)
        wv_a = persist.tile([P, 4, 256], F32R)
        wv_b = persist.tile([P, 4, 256], F32R)
        wo_sb = persist.tile([64, UNITS, D], F32R)  # per-unit Wo rows (K=64)
        ones_sb = persist.tile([P, P], F32R)  # all-ones; row 64 = K=1 lhsT
        # O' accumulator: rows 0-63 unnormalized out^T, row 64 = denominator
        acc = persist.tile([65, QB, PAIRS, 2, 512], F32)
        # hidden^T lives only within its round
        ht_pool = ctx.enter_context(tc.tile_pool(name="htp", bufs=int(os.environ.get("K_HT", "2"))))

        # memset can't write fp32r; stage ones in f32 and round via copies
        ones_f32 = persist.tile([P, P], F32)
        nc.vector.memset(ones_f32, 1.0)
        # identity + weights go on the SWDGE queue so they don't
        # head-block the hidden-tile loads on the HWDGE queue
        # ordered by first use: V projection runs first in each round,
        # Wo isn't needed until the first finalize
        nc.gpsimd.dma_start(wv_a, wvt[:, 0:4, :])
        nc.gpsimd.dma_start(wv_b, wvt[:, 4:8, :])
        nc.gpsimd.dma_start(wk_sb, wkt)
        nc.gpsimd.dma_start(wq_sb, wqt)
        nc.gpsimd.dma_start(wo_sb, wot)
        nc.vector.tensor_copy(ones_sb, ones_f32)
        for col in (64, 129):
            nc.vector.tensor_copy(
                Vp[:, :, :, col : col + 1],
                ones_f32[:, 0:32].rearrange("p (a b c) -> p a b c", a=KI, b=PAIRS),
            )

        def attend_cell(qb, kc, pairs=None):
            """Attention for q-block qb against key chunks 4*kc..4*kc+3."""
            for p in pairs if pairs is not None else range(PAIRS):
                otA = cell_ps.tile([P, 512], F32, tag="ot")
                otB = cell_ps.tile([P, 512], F32, tag="ot")
                for k4 in range(4):
                    ki = kc * 4 + k4
                    stt = st_ps.tile([P, 1024], F32, tag="st")
                    nc.tensor.matmul(
                        stt[:, 0:512],
                        KT[0:64, p, ts(ki, 128)],
                        QT[0:64, p, ts(qb, 512)],
                        start=True,
                        stop=True,
                    )
                    nc.tensor.matmul(
                        stt[:, 512:1024],
                        KT[64:128, p, ts(ki, 128)],
                        QT[64:128, p, ts(qb, 512)],
                        start=True,
                        stop=True,
                    )
                    pt = pt_pool.tile([P, 1024], F32R)
                    nc.scalar.activation(
                        pt, stt, mybir.ActivationFunctionType.Exp, scale=SCALE
                    )
                    nc.tensor.matmul(
                        otA[0:65, :],
                        Vp[:, ki, p, 0:65],
                        pt[:, 0:512],
                        start=(k4 == 0),
                        stop=(k4 == 3),
                    )
                    nc.tensor.matmul(
                        otB[0:65, :],
                        Vp[:, ki, p, 65:130],
                        pt[:, 512:1024],
                        start=(k4 == 0),
                        stop=(k4 == 3),
                    )
                for u, ot in ((0, otA), (1, otB)):
                    sl = acc[:, qb, p, u, :]
                    if kc == 0:
                        nc.vector.tensor_copy(sl, ot[0:65, :])
                    else:
                        nc.vector.tensor_add(sl, sl, ot[0:65, :])

        def finalize(qb):
            """Normalize q-block qb and run its output projection."""
            ot_units = []
            for p in range(PAIRS):
                # one batched reciprocal covers both units of the pair
                rc = rc_pool.tile([65, 1024], F32R)
                with nc.allow_low_precision(
                    reason="fp32r rounding of softmax scale is plenty"
                ):
                    nc.vector.reciprocal(
                        rc[64:65, :],
                        acc[64:65, qb, p, :, :].rearrange("p a f -> p (a f)"),
                    )
                for u in range(2):
                    av = acc[:, qb, p, u, :]
                    rcs = rc[:, u * 512 : (u + 1) * 512]
                    bcp = (st_ps.tile([P, 1024], F32, tag="st", name="trans")[:, 0:512]
                           if os.environ.get("K_TRANS") == "st"
                           else ot_ps.tile([P, 512], F32, tag="ot"))
                    nc.tensor.matmul(
                        bcp, ones_sb[64:65, :], rcs[64:65, :], start=True, stop=True
                    )
                    otu = otsb_pool.tile([64, 512], F32R)
                    nc.vector.tensor_mul(otu, av[0:64, :], bcp[0:64, :])
                    ot_units.append(otu)
            for qt_i in range(4):
                for oh in range(2):
                    yps = (st_ps.tile([P, 1024], F32, tag="st", name="trans")[:, 0:512]
                           if os.environ.get("K_TRANS") == "st"
                           else ot_ps.tile([P, 512], F32, tag="ot"))
                    for u in range(UNITS):
                        nc.tensor.matmul(
                            yps,
                            ot_units[u][:, ts(qt_i, 128)],
                            wo_sb[:, u, ds(oh * 512, 512)],
                            start=(u == 0),
                            stop=(u == UNITS - 1),
                        )
                    ysb = y_pool.tile([P, 512], F32)
                    nc.vector.tensor_copy(ysb, yps)
                    nc.sync.dma_start(
                        y[qb * 512 + qt_i * 128 : qb * 512 + (qt_i + 1) * 128,
                          ds(oh * 512, 512)],
                        ysb,
                    )

        # ---- triangular pipeline: per s-block round, transpose + project,
        # then run every attention cell that just became ready ----
        deferred = []
        for sb in range(SB):
            # two separate half-tiles so the first projection matmuls
            # (dc 0-3) can start as soon as the first half lands (tile
            # pool dependencies are tile-granular)
            hTa = ht_pool.tile([P, 4, 512], F32R, tag="hta")
            hTb = ht_pool.tile([P, 4, 512], F32R, tag="htb")
            nc.sync.dma_start(hTa, hiddent[:, 0:4, ts(sb, 512)])
            nc.sync.dma_start(hTb, hiddent[:, 4:8, ts(sb, 512)])

            def hts(dc):
                return (hTa if dc < 4 else hTb)[:, dc % 4, :]
            for st in range(4):
                s0 = sb * 512 + st * 128
                ki_idx = sb * 4 + st
                # V projection for this s-tile (natural layout, all 4 units)
                vps = (st_ps.tile([P, 1024], F32, tag="st", name="trans")[:, 0:512]
                       if os.environ.get("K_TRANS") == "st"
                       else ot_ps.tile([P, 512], F32, tag="ot"))
                for dc in range(DC):
                    nc.tensor.matmul(
                        vps[:, :256],
                        hts(dc)[:, ts(st, 128)],
                        (wv_a if dc < 4 else wv_b)[:, dc % 4, :],
                        start=(dc == 0),
                        stop=(dc == DC - 1),
                    )
                for p in range(PAIRS):
                    nc.vector.tensor_copy(
                        Vp[:, ki_idx, p, 0:64], vps[:, (2 * p) * 64 : (2 * p + 1) * 64]
                    )
                    nc.vector.tensor_copy(
                        Vp[:, ki_idx, p, 65:129],
                        vps[:, (2 * p + 1) * 64 : (2 * p + 2) * 64],
                    )
            # K^T / Q^T projections for this s-block (pair-packed),
            # interleaved with cells deferred from the previous round so the
            # ACT engine stays fed while the PE runs projections
            projs = [(w, o, p) for (w, o) in ((wk_sb, KT), (wq_sb, QT))
                     for p in range(PAIRS)]
            for i, (w_sb, out_t, p) in enumerate(projs):
                kps = (st_ps.tile([P, 1024], F32, tag="st", name="trans")[:, 0:512]
                       if os.environ.get("K_TRANS") == "st"
                       else ot_ps.tile([P, 512], F32, tag="ot"))
                for dc in range(DC):
                    nc.tensor.matmul(
                        kps,
                        w_sb[:, dc, ts(p, 128)],
                        hts(dc),
                        start=(dc == 0),
                        stop=(dc == DC - 1),
                    )
                nc.vector.tensor_copy(out_t[:, p, ts(sb, 512)], kps)
                if i < len(deferred):
                    attend_cell(*deferred[i])
            deferred = []

            # newly-ready cells: earlier q-blocks against this round's keys,
            # plus this q-block against all keys so far
            new_cells = [(qb, sb) for qb in range(sb)]
            new_cells += [(sb, kc) for kc in range(sb + 1)]
            if sb < SB - 1:
                # defer the last N_DEFER cells, split per pair, to interleave
                # with the next round's projections
                n_defer = min(int(os.environ.get("K_DEFER", "1")), len(new_cells))
                if n_defer:
                    for qb, kc in new_cells[-n_defer:]:
                        for p in range(PAIRS):
                            deferred.append((qb, kc, [p]))
                    deferred = deferred[:4]
                    leftover = [
                        (qb, kc, [p])
                        for (qb, kc) in new_cells[-n_defer:]
                        for p in range(PAIRS)
                    ][4:]
                    new_cells = new_cells[:-n_defer]
                else:
                    leftover = []
                for cell in new_cells:
                    attend_cell(*cell)
                for qb, kc, ps in leftover:
                    attend_cell(qb, kc, ps)
            else:
                # last round: run this q-block's own cells first so its
                # finalize unlocks early, then finalize each q-block one
                # cell after its final cell lands, so finalize PE work
                # fills ACT-wait gaps of the in-flight cell
                if os.environ.get("K_LASTFIRST") == "1":
                    new_cells = new_cells[sb:] + new_cells[:sb]
                done = []
                for i, (qb, kc) in enumerate(new_cells):
                    attend_cell(qb, kc)
                    if done:
                        finalize(done.pop())
                    if (qb, kc) == (qb, SB - 1) and kc == SB - 1:
                        done.append(qb)
                for qb in done:
                    finalize(qb)
    nc.compile()
    return nc


_NC = None


def get_nc():
    global _NC
    if _NC is None:
        _NC = build_nc()
    return _NC


def shard_inputs(hidden_states, Wq, Wk, Wv, Wo):
    """Per-core input maps. Core c: batch c//4, heads 4*(c%4) .. 4*(c%4)+3."""
    hidden_states = np.asarray(hidden_states, np.float32)
    Wq, Wk, Wv, Wo = (np.asarray(w, np.float32) for w in (Wq, Wk, Wv, Wo))
    in_maps = []
    for c in range(N_CORES):
        b = c // 4
        f0 = (c % 4) * 4 * DIM_HEAD  # first feature row/col of this core's heads
        rows = slice(f0, f0 + UNITS * DIM_HEAD)

        def proj_layout(w):
            # W[rows].T is [D, 256]; on-chip layout is [128, DC, 256]
            return np.ascontiguousarray(
                w[rows, :].T.reshape(DC, P, 256).transpose(1, 0, 2)
            )

        # Wo[:, rows].T is [256, D]; on-chip layout is [64, UNITS, D]
        wot = np.ascontiguousarray(
            Wo[:, rows].T.reshape(UNITS, 64, D).transpose(1, 0, 2)
        )
        in_maps.append(
            {
                "hiddent": np.ascontiguousarray(
                    hidden_states[b].T.reshape(DC, P, S).transpose(1, 0, 2)
                ),
                "wqt": proj_layout(Wq),
                "wkt": proj_layout(Wk),
                "wvt": proj_layout(Wv),
                "wot": wot,
            }
        )
    return in_maps


def unshard_outputs(results, bo):
    out = np.zeros((B, S, D), np.float32)
    for c, res in enumerate(results):
        out[c // 4] += res["y"]
    out += np.asarray(bo, np.float32)[None, None, :]
    return out


def kernel(hidden_states, Wq, Wk, Wv, Wo, bo, _trace=False):
    from concourse.bass_utils import run_bass_kernel_spmd

    nc = get_nc()
    in_maps = shard_inputs(hidden_states, Wq, Wk, Wv, Wo)
    res = run_bass_kernel_spmd(nc, in_maps, list(range(N_CORES)), trace=_trace)
    out = unshard_outputs(res.results, bo)
    if _trace:
        return out, res
    return out

